# revision 27
# baseline (speedup 1.0000x reference)
"""AFT-Full (nn_AFT_Full) Trainium2 Bass kernel, 8-core SPMD, batch-sharded.

Math note: in the reference, w_bias has shape [1,T,T] and max over dim 0 is the
identity, so exp_wb == exp(0) == 1 and the [T,T] matmuls reduce to column sums
over T (u/vp are unused):
    num[b,h] = sum_t exp(k[b,t,h] - m[t,h]) * v[b,t,h]
    den[b,h] = sum_t exp(k[b,t,h] - m[t,h])
    out = (sigmoid(q) * num/den) @ Wo + bo
where m = max over the FULL batch of k -> cross-core AllReduce(max).

We compute E0 = exp(k + bk) directly (fused into the PSUM->SBUF copy), take
M = max_b E0 (exp is monotone, so this is exp(m)), AllReduce(max) on M, and use
s = 1/M so that exp(k - m) == E0 * s.

Schedule: two passes over x (k,v then q) with on-chip TE transposes; the
batch-max M is accumulated per window during pass 1 so the collective fires
immediately after; pass-2 x loads are issued before the collective trigger
(gpsimd is in-order) and num/den chunks interleave with pass-2 windows.
"""
import os
import sys

sys.path.insert(0, "/opt/trn_rl_repo")

import numpy as np

# ---- problem constants (hardcoded per spec) ----
B, Hh, Ww, C = 64, 24, 24, 768
HID = 576
T = Hh * Ww          # 576
N_CORES = 8
B_LOC = B // N_CORES  # 8
R = B_LOC * T         # 4608 rows per core
WIN = 512             # row window
NWIN = R // WIN       # 9
NRC = WIN // 128      # 4 row chunks per window
NCC = C // 128        # 6 contraction chunks for projections
HC_SIZES = [128, 128, 128, 128, 64]   # HID = 576 partition chunks
NOUT_HALF = 384       # out matmul free-dim split (768 = 2*384)

_CACHE = {}
LAST_EXEC_NS = None


def _window_segments(w):
    """Batch segments [(b, lo, hi)] of window w, window-local coords."""
    lo, hi = w * WIN, (w + 1) * WIN
    segs = []
    for b in range(B_LOC):
        s = max(lo, b * T)
        e = min(hi, (b + 1) * T)
        if s < e:
            segs.append((b, s - lo, e - lo))
    return segs


def _build():
    import concourse.bass as bass
    import concourse.mybir as mybir
    from concourse import bacc, tile

    f32 = mybir.dt.float32
    bf16 = mybir.dt.bfloat16
    AF = mybir.ActivationFunctionType

    nc = bacc.Bacc("TRN2", target_bir_lowering=False, debug=False,
                   num_devices=N_CORES)

    x = nc.dram_tensor("x", [R, C], f32, kind="ExternalInput").ap()
    Wq = nc.dram_tensor("Wq", [C, HID], f32, kind="ExternalInput").ap()
    Wk = nc.dram_tensor("Wk", [C, HID], f32, kind="ExternalInput").ap()
    Wv = nc.dram_tensor("Wv", [C, HID], f32, kind="ExternalInput").ap()
    bq = nc.dram_tensor("bq", [HID], f32, kind="ExternalInput").ap()
    bk = nc.dram_tensor("bk", [HID], f32, kind="ExternalInput").ap()
    bv = nc.dram_tensor("bv", [HID], f32, kind="ExternalInput").ap()
    Wo = nc.dram_tensor("Wo", [HID, C], f32, kind="ExternalInput").ap()
    bo = nc.dram_tensor("bo", [C], f32, kind="ExternalInput").ap()
    ident = nc.dram_tensor("ident", [128, 128], bf16, kind="ExternalInput").ap()
    out = nc.dram_tensor("out", [R, C], f32, kind="ExternalOutput").ap()

    with tile.TileContext(nc) as tc:
        with (
            tc.tile_pool(name="const", bufs=1) as cpool,
            tc.tile_pool(name="resident", bufs=1) as rpool,
            tc.tile_pool(name="xn", bufs=2) as xnpool,
            tc.tile_pool(name="xt", bufs=3) as xtpool,
            tc.tile_pool(name="qy", bufs=6) as qypool,
            tc.tile_pool(name="ob", bufs=2) as obpool,
            tc.tile_pool(name="sc", bufs=1) as scpool,
            tc.tile_pool(name="pt", bufs=2, space="PSUM") as ptpool,
            tc.tile_pool(name="pm", bufs=3, space="PSUM") as pmpool,
            tc.tile_pool(name="po", bufs=3, space="PSUM") as popool,
            tc.tile_pool(name="dram", bufs=1, space="DRAM") as dpool,
        ):
            # ---------- constants ----------
            ident_sb = cpool.tile([128, 128], bf16, tag="ident", name="ident")
            nc.sync.dma_start(ident_sb[:], ident[:])

            def load_w(name, w_ap):
                tiles = []
                for cc in range(NCC):
                    t = cpool.tile([128, HID], bf16, tag=f"{name}_{cc}",
                                   name=f"{name}_{cc}")
                    nc.gpsimd.dma_start(t[:], w_ap[cc * 128:(cc + 1) * 128, :])
                    tiles.append(t)
                return tiles


            def load_wo():
                # Wo extended with bo as an extra contraction row (ones trick)
                tiles = []
                for kc, ksz in enumerate(HC_SIZES):
                    psz = ksz + 1 if kc == 4 else ksz
                    t = cpool.tile([psz, C], bf16, tag=f"Wo_{kc}",
                                   name=f"Wo_{kc}")
                    nc.gpsimd.dma_start(t[0:ksz, :],
                                        Wo[kc * 128:kc * 128 + ksz, :])
                    if kc == 4:
                        nc.gpsimd.dma_start(t[ksz:ksz + 1, :], bo[None, :])
                    tiles.append(t)
                return tiles

            def load_bias(name, b_ap):
                tiles = []
                for hc, hsz in enumerate(HC_SIZES):
                    t = cpool.tile([hsz, 1], f32, tag=f"{name}_{hc}",
                                   name=f"{name}_{hc}")
                    nc.sync.dma_start(t[:], b_ap[hc * 128:hc * 128 + hsz][:, None])
                    tiles.append(t)
                return tiles

            bk_sb = load_bias("bk", bk)
            bv_sb = load_bias("bv", bv)
            bq_sb = load_bias("bq", bq)

            # ---------- resident tensors ----------
            E0 = [rpool.tile([hsz, R], bf16, tag=f"E0_{hc}", name=f"E0_{hc}")
                  for hc, hsz in enumerate(HC_SIZES)]
            Vs = [rpool.tile([hsz, R], bf16, tag=f"V_{hc}", name=f"V_{hc}")
                  for hc, hsz in enumerate(HC_SIZES)]
            Mx = [rpool.tile([hsz, T], bf16, tag=f"M_{hc}", name=f"M_{hc}")
                  for hc, hsz in enumerate(HC_SIZES)]
            den = [rpool.tile([hsz, B_LOC], f32, tag=f"den_{hc}", name=f"den_{hc}")
                   for hc, hsz in enumerate(HC_SIZES)]
            num = [rpool.tile([hsz, B_LOC], f32, tag=f"num_{hc}", name=f"num_{hc}")
                   for hc, hsz in enumerate(HC_SIZES)]
            rr = [rpool.tile([hsz, B_LOC], f32, tag=f"r_{hc}", name=f"r_{hc}")
                  for hc, hsz in enumerate(HC_SIZES)]

            xt_dram = dpool.tile([NWIN, 128, NCC * WIN], bf16, name="xt_dram")

            def load_xn(w):
                xn = xnpool.tile([128, NRC * C], bf16, tag="xn", name="xn")
                src = x[w * WIN:(w + 1) * WIN, :].rearrange(
                    "(n p) c -> p n c", p=128)
                nc.gpsimd.dma_start(
                    xn[:].rearrange("p (n c) -> p n c", c=C), src)
                return xn

            def transpose_xt(xn):
                """xt[c_part, cc*WIN + r]; 4 transposes batched per psum tile,
                one DVE copy per cc."""
                xt = xtpool.tile([128, NCC * WIN], bf16, tag="xt", name="xt")
                for cc in range(NCC):
                    pt = ptpool.tile([128, WIN], bf16, tag="pt", name="pt")
                    for rc in range(NRC):
                        nc.tensor.transpose(
                            pt[:, rc * 128:(rc + 1) * 128],
                            xn[:, rc * C + cc * 128: rc * C + (cc + 1) * 128],
                            ident_sb[:])
                    nc.vector.tensor_copy(
                        xt[:, cc * WIN:(cc + 1) * WIN], pt[:])
                return xt

            def project(xt, w_tiles, hc, hsz):
                pm = pmpool.tile([hsz, WIN], f32, tag="pm", name="pm")
                for cc in range(NCC):
                    nc.tensor.matmul(
                        pm[:],
                        w_tiles[cc][:, hc * 128: hc * 128 + hsz],
                        xt[:, cc * WIN:(cc + 1) * WIN],
                        start=(cc == 0), stop=(cc == NCC - 1))
                return pm

            # ---------- pass 1: k (as exp) and v; M accumulated per window ----
            xns = {0: load_xn(0)}
            xts = {0: transpose_xt(xns[0])}
            Wk_sb = load_w("Wk", Wk)
            xns[1] = load_xn(1)
            xts[1] = transpose_xt(xns[1])
            Wv_sb = load_w("Wv", Wv)
            xns[2] = load_xn(2)
            xts[2] = transpose_xt(xns[2])
            Wq_sb = load_w("Wq", Wq)
            xns[3] = load_xn(3)
            xts[3] = transpose_xt(xns[3])
            Wo_sb = load_wo()

            def mx_acc(w):
                # batch-max accumulation (b==0 initializes, else running max)
                for b, lo, hi in _window_segments(w):
                    t0 = w * WIN + lo - b * T
                    t1 = t0 + (hi - lo)
                    for hc, hsz in enumerate(HC_SIZES):
                        e_seg = E0[hc][:, w * WIN + lo: w * WIN + hi]
                        if b == 0:
                            nc.vector.tensor_copy(Mx[hc][:, t0:t1], e_seg)
                        else:
                            nc.vector.tensor_max(
                                Mx[hc][:, t0:t1], Mx[hc][:, t0:t1], e_seg)

            for w in range(NWIN):
                nc.sync.dma_start(xt_dram[w], xts[w][:])
                if w + 1 < NWIN and w + 1 not in xns:
                    xns[w + 1] = load_xn(w + 1)
                    xts[w + 1] = transpose_xt(xns[w + 1])
                xt = xts[w]
                for hc, hsz in enumerate(HC_SIZES):
                    pm = project(xt, Wk_sb, hc, hsz)
                    nc.scalar.activation(
                        E0[hc][:, w * WIN:(w + 1) * WIN], pm[:],
                        AF.Exp, bias=bk_sb[hc][:])
                for hc, hsz in enumerate(HC_SIZES):
                    pm = project(xt, Wv_sb, hc, hsz)
                    nc.scalar.activation(
                        Vs[hc][:, w * WIN:(w + 1) * WIN], pm[:],
                        AF.Identity, bias=bv_sb[hc][:])
                mx_acc(w)

            # ---------- s_loc = 1/M_loc (pre-collective, scalar engine);
            # AllReduce(min) of s == 1/AllReduce(max) of M. Split in two:
            # s[:, 0:TSPLIT] is final after window 7, so its (big) collective
            # fires one window early; the tail goes in a second small one.
            Sx = Mx  # s = 1/M computed in place (exp(-ln(M)))
            bounce_in = dpool.tile([HID, T], bf16, name="bounce_in")
            bounce_out = dpool.tile([HID, T], bf16, name="bounce_out",
                                    addr_space="Shared")

            def s_chunk(t0, t1):
                for hc, hsz in enumerate(HC_SIZES):
                    lnm = scpool.tile([hsz, t1 - t0], f32, tag="lnm",
                                      name="lnm")
                    nc.scalar.activation(lnm[:], Mx[hc][:, t0:t1], AF.Ln)
                    nc.scalar.activation(Sx[hc][:, t0:t1], lnm[:], AF.Exp,
                                         scale=-1.0)
                    nc.sync.dma_start(
                        bounce_in[hc * 128:hc * 128 + hsz, t0:t1],
                        Sx[hc][:, t0:t1])

            def s_collective(t0, t1):
                nc.gpsimd.collective_compute(
                    "AllReduce",
                    mybir.AluOpType.min,
                    replica_groups=[list(range(N_CORES))],
                    ins=[bounce_in[:, t0:t1].opt()],
                    outs=[bounce_out[:, t0:t1].opt()],
                )
                for hc, hsz in enumerate(HC_SIZES):
                    nc.gpsimd.dma_start(
                        Sx[hc][:, t0:t1],
                        bounce_out[hc * 128:hc * 128 + hsz, t0:t1])

            s_chunk(0, T)

            # prefetch first pass-2 xt reads on sync BEFORE the collectives
            # are traced: both sync and gpsimd queues are in-order, and the
            # readbacks (which must wait for the AR) live on gpsimd so the
            # sync queue never blocks on the collective.
            def read_xt(w):
                xt = xtpool.tile([128, NCC * WIN], bf16, tag="xt", name="xt")
                nc.sync.dma_start(xt[:], xt_dram[w])
                return xt

            xt2 = {0: read_xt(0), 1: read_xt(1), 2: read_xt(2)}

            s_collective(0, T)

            def nd_batch(b):
                """num/den/r for one batch across all h chunks."""
                for hc, hsz in enumerate(HC_SIZES):
                    e_b = E0[hc][:, b * T:(b + 1) * T]
                    v_b = Vs[hc][:, b * T:(b + 1) * T]
                    d_b = den[hc][:, b:b + 1]
                    n_b = num[hc][:, b:b + 1]
                    r_b = rr[hc][:, b:b + 1]
                    nc.vector.tensor_mul(e_b, e_b, Sx[hc][:])
                    nc.vector.reduce_sum(d_b, e_b, axis=mybir.AxisListType.X)
                    nc.vector.tensor_mul(e_b, e_b, v_b)
                    nc.vector.reduce_sum(n_b, e_b, axis=mybir.AxisListType.X)
                    nc.vector.reciprocal(r_b, d_b)
                    nc.vector.tensor_mul(r_b, r_b, n_b)

            # ---------- pass 2: q -> sigmoid -> y -> out, nd interleaved ----
            for w in range(NWIN):
                if w + 3 < NWIN:
                    xt2[w + 3] = read_xt(w + 3)
                for b in range(B_LOC):
                    if (b * T) // WIN == w:
                        nd_batch(b)
                xt = xt2[w]
                qy = []
                for hc, hsz in enumerate(HC_SIZES):
                    psz = hsz + 1 if hc == 4 else hsz
                    t = qypool.tile([psz, WIN], bf16, tag=f"qy_{hc}",
                                    name=f"qy_{hc}")
                    pm = project(xt, Wq_sb, hc, hsz)
                    nc.scalar.activation(
                        t[0:hsz, :], pm[:], AF.Sigmoid, bias=bq_sb[hc][:])
                    if hc == 4:
                        nc.vector.memset(t[hsz:hsz + 1, :], 1.0)
                    qy.append(t)
                # y = sigmoid(q) * r  (r constant over t within one batch)
                for b, lo, hi in _window_segments(w):
                    for hc, hsz in enumerate(HC_SIZES):
                        nc.vector.tensor_scalar_mul(
                            qy[hc][0:hsz, lo:hi],
                            qy[hc][0:hsz, lo:hi],
                            rr[hc][:, b:b + 1])
                # out = y_ext @ Wo_ext
                for rc in range(NRC):
                    ob = obpool.tile([128, C], f32, tag="ob", name="ob")
                    poa = popool.tile([128, NOUT_HALF], f32, tag="po", name="po")
                    pob = popool.tile([128, NOUT_HALF], f32, tag="po", name="po")
                    for kc, ksz in enumerate(HC_SIZES):
                        psz = ksz + 1 if kc == 4 else ksz
                        lhs = qy[kc][0:psz, rc * 128:(rc + 1) * 128]
                        nc.tensor.matmul(
                            poa[:], lhs, Wo_sb[kc][0:psz, 0:NOUT_HALF],
                            start=(kc == 0), stop=(kc == 4))
                        nc.tensor.matmul(
                            pob[:], lhs, Wo_sb[kc][0:psz, NOUT_HALF:C],
                            start=(kc == 0), stop=(kc == 4))
                    nc.scalar.copy(ob[:, 0:NOUT_HALF], poa[:])
                    nc.scalar.copy(ob[:, NOUT_HALF:C], pob[:])
                    nc.sync.dma_start(
                        out[w * WIN + rc * 128: w * WIN + (rc + 1) * 128, :],
                        ob[:])

    nc.compile()
    return nc


def kernel(**inputs):
    global LAST_EXEC_NS
    from concourse import bass_utils

    if "nc" not in _CACHE:
        _CACHE["nc"] = _build()
    nc = _CACHE["nc"]

    x = np.asarray(inputs["x"], dtype=np.float32).reshape(B, T, C)
    import ml_dtypes
    eye = np.eye(128, dtype=ml_dtypes.bfloat16)
    common = {
        "Wq": np.asarray(inputs["Wq"], np.float32),
        "Wk": np.asarray(inputs["Wk"], np.float32),
        "Wv": np.asarray(inputs["Wv"], np.float32),
        "bq": np.asarray(inputs["bq"], np.float32),
        "bk": np.asarray(inputs["bk"], np.float32),
        "bv": np.asarray(inputs["bv"], np.float32),
        "Wo": np.asarray(inputs["Wo"], np.float32),
        "bo": np.asarray(inputs["bo"], np.float32),
        "ident": eye,
    }
    in_maps = []
    for i in range(N_CORES):
        m = dict(common)
        m["x"] = np.ascontiguousarray(
            x[i * B_LOC:(i + 1) * B_LOC].reshape(R, C))
        in_maps.append(m)

    trace = bool(os.environ.get("KERNEL_TRACE"))
    res = bass_utils.run_bass_kernel_spmd(
        nc, in_maps, core_ids=list(range(N_CORES)), trace=trace)
    LAST_EXEC_NS = res.exec_time_ns

    shards = [res.results[i]["out"].reshape(B_LOC, Hh, Ww, C)
              for i in range(N_CORES)]
    return np.concatenate(shards, axis=0)
